# revision 1
# baseline (speedup 1.0000x reference)
"""DA3CrossFrameRKDAngleLoss Trainium2 kernel (bass/Tile).  v4

Sharding: 8 cores = (batch b = core//2) x (ref-row half = core%2).
Each core handles R=128 ref rows of one batch; host sums partial sums.

Per-core math (R=128, S=256, K=4, D=1024, E=4096, RK=R*K=512; rk = r*4+k):
  sim[r,e] = ref_t[r] . extra_unit[e]   fp32r (row scale irrelevant to topk)
  top4 per row -> sim_high[rk] = extra[idx[rk]]  (raw rows, bf16 gather)
  rr[rk] (host)   rh[rk] = ref.sim   hh[rk] = |sim|^2   (device)
  sr'[r,(f,s)] = ref.shared          (bf16 MMs, frames merged N=768)
  T[r,s]   = -2*sr' + (rr+ss)        (DVE stt fold, W=rr+ss uploaded)
  P1[rk,s] = sim.shared - sr'_rep    (bf16 MMs + bf16 R4 psum accumulation)
  T_rep, q_rep: f32r R4 matmuls -> fp16 SBUF copies
  angle elementwise in fp16 (DVE 2x), reciprocals fp32
  a1 = (P1 + rr - rh) * q_hr * q_sr
  a2 = (-P1 + hh - rh) * q_hr * q_sh
  a3 = (T_rep - P1 + rh - rr) * q_sr * q_sh;  n_sh^2 = T_rep - 2*P1 + (hh - rr)
  acc[f,a] = sum |a_teacher - a_student|
loss = sum(acc over all cores) / (3*B*256*256*4)
"""
import sys
sys.path.insert(0, '/opt/trn_rl_repo')
import numpy as np
import ml_dtypes

import concourse.bass as bass
import concourse.mybir as mybir
import concourse.tile as tile
from concourse import bacc
from concourse.bass_utils import run_bass_kernel_spmd

AF = mybir.ActivationFunctionType
OP = mybir.AluOpType
F32 = mybir.dt.float32
F32R = mybir.dt.float32r
BF16 = mybir.dt.bfloat16
F16 = mybir.dt.float16

R, S, K, D, E = 128, 256, 4, 1024, 4096
RK = R * K
NF = 3
KC = D // 128          # 8
ET = E // 512          # 8


def build_program(n_cores=8):
    nc = bacc.Bacc("TRN2", target_bir_lowering=False, debug=False,
                   num_devices=n_cores)
    d = {}
    d['extra_nt'] = nc.dram_tensor("extra_nt", [D, E], F32R, kind="ExternalInput").ap()
    d['reft_t'] = nc.dram_tensor("reft_t", [D, R], F32R, kind="ExternalInput").ap()
    d['reft_bf'] = nc.dram_tensor("reft_bf", [2, D, R], BF16, kind="ExternalInput").ap()
    d['sh_bf'] = nc.dram_tensor("sh_bf", [NF, D, 2 * S], BF16, kind="ExternalInput").ap()
    d['w_rs'] = nc.dram_tensor("w_rs", [NF, 2, 128, S], F32, kind="ExternalInput").ap()
    d['extra_bf'] = nc.dram_tensor("extra_bf", [E, D], BF16, kind="ExternalInput").ap()
    d['refn_bf'] = nc.dram_tensor("refn_bf", [2, RK, D], BF16, kind="ExternalInput").ap()
    d['rr'] = nc.dram_tensor("rr", [2, 128, K], F32, kind="ExternalInput").ap()
    d['r4'] = nc.dram_tensor("r4", [128, RK], F32R, kind="ExternalInput").ap()
    d['r4_bf'] = nc.dram_tensor("r4_bf", [128, RK], BF16, kind="ExternalInput").ap()
    d['acc'] = nc.dram_tensor("acc", [128, NF * 3], F32, kind="ExternalOutput").ap()
    d['idx'] = nc.dram_tensor("idx", [128, 8], mybir.dt.uint32, kind="ExternalOutput").ap()

    with tile.TileContext(nc) as tc:
        _body(nc, tc, d)
    nc.compile()
    return nc


def _body(nc, tc, d):
    from contextlib import ExitStack
    with ExitStack() as ctx:
        sb = ctx.enter_context(tc.tile_pool(name="persist", bufs=1))

        # ---- resident tiles; sync queue: phase-critical, scalar queue: later ----
        sh = [sb.tile([128, KC, 2 * S], BF16, tag=f"sh{f}", name=f"sh{f}") for f in range(NF)]
        w_rs = [[sb.tile([128, S], F32, tag=f"w{f}{n}", name=f"w{f}{n}")
                 for n in range(2)] for f in range(NF)]
        refn = [sb.tile([128, K, D], BF16, tag=f"refn{n}", name=f"refn{n}") for n in range(2)]
        rr = [sb.tile([128, K], F32, tag=f"rr{n}", name=f"rr{n}") for n in range(2)]
        r4 = sb.tile([128, RK], F32R, tag="r4", name="r4")
        r4b = sb.tile([128, RK], BF16, tag="r4b", name="r4b")
        for f in range(NF):
            nc.sync.dma_start(sh[f][:], d['sh_bf'][f].rearrange("(c p) s -> p c s", p=128))
        for f in range(NF):
            for n in range(2):
                nc.scalar.dma_start(w_rs[f][n][:], d['w_rs'][f, n])
        for n in range(2):
            nc.scalar.dma_start(refn[n][:], d['refn_bf'][n].rearrange("(g p) x -> p g x", p=128))
            nc.scalar.dma_start(rr[n][:], d['rr'][n])
        nc.scalar.dma_start(r4[:], d['r4'])
        nc.scalar.dma_start(r4b[:], d['r4_bf'])

        sim_hi = sb.tile([128, K, D], BF16, tag="sim_hi", name="sim_hi")
        simT = sb.tile([128, KC, RK], BF16, tag="simT", name="simT")
        T_sb = [[None] * 2 for _ in range(NF)]
        qsr_sb = [[None] * 2 for _ in range(NF)]
        nsrp_bf = [None] * NF
        acc = sb.tile([128, NF * 3], F32, tag="acc", name="acc")

        with tc.tile_pool(name="early", bufs=1) as eb:
            reft = eb.tile([128, KC, R], F32R, tag="reft", name="reft")
            refb = [eb.tile([128, KC, R], BF16, tag=f"refb{n}", name=f"refb{n}") for n in range(2)]
            nc.sync.dma_start(reft[:], d['reft_t'].rearrange("(c p) r -> p c r", p=128))
            for n in range(2):
                nc.sync.dma_start(refb[n][:], d['reft_bf'][n].rearrange("(c p) r -> p c r", p=128))
            sim_sb = eb.tile([128, E], F32, tag="sim_sb", name="sim_sb")

            # ---- phase 2 FIRST: sr' (frames merged), T, q_sr per net ----
            with tc.tile_pool(name="srps", bufs=2, space="PSUM") as srps:
                for n in range(2):
                    sp3 = srps.tile([128, NF, S], F32, tag="sp3", name="sp3")
                    for f in range(NF):
                        for kc in range(KC):
                            nc.tensor.matmul(sp3[:, f, :], refb[n][:, kc, :],
                                             sh[f][:, kc, n * S:(n + 1) * S],
                                             start=(kc == 0), stop=(kc == KC - 1))
                    for f in range(NF):
                        T_sb[f][n] = sb.tile([128, S], F32R, tag=f"T{f}{n}", name=f"T{f}{n}")
                        nc.vector.scalar_tensor_tensor(out=T_sb[f][n][:],
                                                       in0=sp3[:, f, :], scalar=-2.0,
                                                       in1=w_rs[f][n][:],
                                                       op0=OP.mult, op1=OP.add)
                        nsr = eb.tile([128, S], F32, tag="nsr_tmp", name="nsr_tmp")
                        nc.scalar.activation(nsr[:], T_sb[f][n][:], AF.Sqrt, bias=0.0)
                        qtmp = eb.tile([128, S], F32, tag="q_tmp", name="q_tmp")
                        nc.vector.reciprocal_approx_fast(out=qtmp[:], in_=nsr[:])
                        qsr_sb[f][n] = sb.tile([128, S], F32R, tag=f"qsr{f}{n}", name=f"qsr{f}{n}")
                        nc.vector.tensor_copy(out=qsr_sb[f][n][:], in_=qtmp[:])
                        if nsrp_bf[f] is None:
                            nsrp_bf[f] = sb.tile([128, 2, S], BF16, tag=f"nsrp{f}", name=f"nsrp{f}")
                        nc.scalar.activation(nsrp_bf[f][:, n, :], sp3[:, f, :],
                                             AF.Copy, scale=-1.0)

            # ---- phase 1: sim (et-pairs, [128,1024] DMA tiles) ----
            with tc.tile_pool(name="ext", bufs=6) as extp, \
                 tc.tile_pool(name="simps", bufs=4, space="PSUM") as simps:
                for e2 in range(ET // 2):
                    psa = simps.tile([128, 512], F32, tag="simps", name="simps_a")
                    psb = simps.tile([128, 512], F32, tag="simps", name="simps_b")
                    for kc in range(KC):
                        x = extp.tile([128, 1024], F32R, tag="ext", name="ext")
                        nc.sync.dma_start(x[:], d['extra_nt'][kc * 128:(kc + 1) * 128,
                                                              e2 * 1024:(e2 + 1) * 1024])
                        nc.tensor.matmul(psa[:], reft[:, kc, :], x[:, 0:512],
                                         start=(kc == 0), stop=(kc == KC - 1))
                        nc.tensor.matmul(psb[:], reft[:, kc, :], x[:, 512:1024],
                                         start=(kc == 0), stop=(kc == KC - 1))
                    nc.scalar.copy(sim_sb[:, e2 * 1024:e2 * 1024 + 512], psa[:])
                    nc.scalar.copy(sim_sb[:, e2 * 1024 + 512:(e2 + 1) * 1024], psb[:])

            # ---- phase 3: topk + gathers ----
            mx = eb.tile([128, 8], F32, tag="mx", name="mx")
            mi = eb.tile([128, 8], mybir.dt.uint32, tag="mi", name="mi")
            nc.vector.max(out=mx[:], in_=sim_sb[:])
            nc.vector.max_index(out=mi[:], in_max=mx[:], in_values=sim_sb[:])
            nc.sync.dma_start(d['idx'][:], mi[:])
            idx16 = eb.tile([128, K], mybir.dt.int16, tag="idx16", name="idx16")
            nc.vector.tensor_copy(out=idx16[:], in_=mi[:, 0:K])

            with tc.tile_pool(name="dram", bufs=1, space="DRAM") as drp:
                idx_dram = drp.tile([RK], mybir.dt.int16, name="idx_dram")
                nc.scalar.dma_start(idx_dram[:].rearrange("(p a) -> p a", p=128),
                                    idx16[:])
                idxw = eb.tile([128, RK // 16], mybir.dt.int16, tag="idxw", name="idxw")
                wrapped = idx_dram[:].rearrange("(j q) -> q j", q=16)
                for sg in range(8):
                    eng = (nc.scalar, nc.sync)[sg % 2]
                    eng.dma_start(idxw[16 * sg:16 * (sg + 1), :], wrapped)
                nc.gpsimd.dma_gather(simT[:], d['extra_bf'], idxw[:], RK, RK, D,
                                     transpose=True, queue_num=0)
                nc.gpsimd.dma_gather(sim_hi[:], d['extra_bf'], idxw[:], RK, RK, D,
                                     queue_num=0)

        # ---- phase 4: per-(rk) scalars ----
        hh = sb.tile([128, K], F32, tag="hh", name="hh")
        dump = sb.tile([128, D], BF16, tag="dump", name="dump")
        for g in range(K):
            nc.scalar.activation(dump[:], sim_hi[:, g, :], AF.Square,
                                 accum_out=hh[:, g:g + 1])
        rh = [sb.tile([128, K], F32, tag=f"rh{n}", name=f"rh{n}") for n in range(2)]
        for n in range(2):
            for g in range(K):
                nc.vector.scalar_tensor_tensor(out=dump[:], in0=sim_hi[:, g, :],
                                               scalar=0.0, in1=refn[n][:, g, :],
                                               op0=OP.bypass, op1=OP.mult,
                                               accum_out=rh[n][:, g:g + 1])
        b1 = [None] * 2; b2 = [None] * 2; b3 = [None] * 2; bsh = [None] * 2
        qhr = [None] * 2; qhrn = [None] * 2
        for n in range(2):
            u1 = sb.tile([128, K], F32, tag=f"u1{n}", name=f"u1{n}")
            u2 = sb.tile([128, K], F32, tag=f"u2{n}", name=f"u2{n}")
            nc.vector.tensor_sub(out=u1[:], in0=rr[n][:], in1=rh[n][:])
            nc.vector.tensor_sub(out=u2[:], in0=hh[:], in1=rh[n][:])
            nhr = sb.tile([128, K], F32, tag=f"nhr{n}", name=f"nhr{n}")
            nc.vector.tensor_add(out=nhr[:], in0=u1[:], in1=u2[:])
            nc.scalar.activation(nhr[:], nhr[:], AF.Sqrt, bias=0.0)
            qhr[n] = sb.tile([128, K], F32, tag=f"qhr{n}", name=f"qhr{n}")
            nc.vector.reciprocal_approx_fast(out=qhr[n][:], in_=nhr[:])
            qhrn[n] = sb.tile([128, K], F32, tag=f"qhrn{n}", name=f"qhrn{n}")
            nc.vector.tensor_scalar_mul(qhrn[n][:], qhr[n][:], -1.0)
            b1[n] = sb.tile([128, K], F32, tag=f"b1{n}", name=f"b1{n}")
            nc.vector.tensor_mul(out=b1[n][:], in0=u1[:], in1=qhr[n][:])
            b2[n] = sb.tile([128, K], F32, tag=f"b2{n}", name=f"b2{n}")
            nc.vector.tensor_mul(out=b2[n][:], in0=u2[:], in1=qhr[n][:])
            b3[n] = sb.tile([128, K], F32, tag=f"b3{n}", name=f"b3{n}")
            nc.vector.tensor_scalar_mul(b3[n][:], u1[:], -1.0)
            bsh[n] = sb.tile([128, K], F32, tag=f"bsh{n}", name=f"bsh{n}")
            nc.vector.tensor_sub(out=bsh[n][:], in0=u2[:], in1=u1[:])

        # ---- phases 5-6: per frame ----
        with tc.tile_pool(name="p1ps", bufs=1, space="PSUM") as p1p, \
             tc.tile_pool(name="repps", bufs=2, space="PSUM") as repp, \
             tc.tile_pool(name="reph", bufs=2) as reph_pool, \
             tc.tile_pool(name="ang", bufs=3) as ang, \
             tc.tile_pool(name="angb", bufs=2) as angb:

            def emit_reps(f):
                out = []
                for n in range(2):
                    trep = repp.tile([128, K, S], F32, tag="rep", name="trep")
                    for g in range(K):
                        nc.tensor.matmul(trep[:, g, :], r4[:, g * 128:(g + 1) * 128],
                                         T_sb[f][n][:], start=True, stop=True)
                    treph = reph_pool.tile([128, K, S], F16, tag=f"treph{n}",
                                           name=f"treph{n}")
                    nc.scalar.copy(treph[:], trep[:])
                    qrep = repp.tile([128, K, S], F32, tag="rep", name="qrep")
                    for g in range(K):
                        nc.tensor.matmul(qrep[:, g, :], r4[:, g * 128:(g + 1) * 128],
                                         qsr_sb[f][n][:], start=True, stop=True)
                    qreph = reph_pool.tile([128, K, S], F16, tag=f"qreph{n}",
                                           name=f"qreph{n}")
                    nc.scalar.copy(qreph[:], qrep[:])
                    out.append((treph, qreph))
                return out

            reps = emit_reps(0)
            for f in range(NF):
                p1 = p1p.tile([128, K, 2 * S], F32, tag="p1", name="p1")
                for kc in range(KC):
                    for g in range(K):
                        nc.tensor.matmul(p1[:, g, :],
                                         simT[:, kc, g * 128:(g + 1) * 128],
                                         sh[f][:, kc, :],
                                         start=(kc == 0), stop=False)
                for g in range(K):
                    nc.tensor.matmul(p1[:, g, :],
                                     r4b[:, g * 128:(g + 1) * 128],
                                     nsrp_bf[f][:].rearrange("p a b -> p (a b)"),
                                     start=False, stop=True)
                cur_reps = reps
                if f + 1 < NF:
                    reps = emit_reps(f + 1)
                a_out = [[None] * 2 for _ in range(3)]
                for n in range(2):
                    treph, qreph = cur_reps[n]
                    p1s = p1[:, :, n * S:(n + 1) * S]          # [128, K, S] psum view
                    a1 = angb.tile([128, K, S], F16, tag=f"a1_{n}", name=f"a1_{n}")
                    a2 = angb.tile([128, K, S], F16, tag=f"a2_{n}", name=f"a2_{n}")
                    a3 = angb.tile([128, K, S], F16, tag=f"a3_{n}", name=f"a3_{n}")
                    # ta1/ta2 per-g (ACT per-partition scale/bias)
                    ta1 = ang.tile([128, K, S], F16, tag="ta1", name="ta1")
                    ta2 = ang.tile([128, K, S], F16, tag="ta2", name="ta2")
                    for g in range(K):
                        nc.scalar.activation(ta1[:, g, :], p1[:, g, n * S:(n + 1) * S],
                                             AF.Identity, scale=qhr[n][:, g:g + 1],
                                             bias=b1[n][:, g:g + 1])
                        nc.scalar.activation(ta2[:, g, :], p1[:, g, n * S:(n + 1) * S],
                                             AF.Identity, scale=qhrn[n][:, g:g + 1],
                                             bias=b2[n][:, g:g + 1])
                    # t5/t6 full-span from psum
                    t5 = ang.tile([128, K, S], F16, tag="t5", name="t5")
                    nc.vector.scalar_tensor_tensor(out=t5[:], in0=p1s, scalar=-1.0,
                                                   in1=treph[:], op0=OP.mult, op1=OP.add)
                    t6 = ang.tile([128, K, S], F16, tag="t6", name="t6")
                    nc.vector.scalar_tensor_tensor(out=t6[:], in0=p1s, scalar=-2.0,
                                                   in1=treph[:], op0=OP.mult, op1=OP.add)
                    # n_sh = sqrt(t6 + bsh)  (bsh broadcast along s on DVE, sqrt full on ACT)
                    t6b = ang.tile([128, K, S], F16, tag="t6b", name="t6b")
                    bsh_bc = bsh[n][:].unsqueeze(2).broadcast_to([128, K, S])
                    nc.vector.tensor_add(out=t6b[:], in0=t6[:], in1=bsh_bc)
                    nsh = ang.tile([128, K, S], F32, tag="nsh", name="nsh")
                    nc.scalar.activation(nsh[:], t6b[:], AF.Sqrt, bias=0.0)
                    qshf = ang.tile([128, K, S], F32, tag="qshf", name="qshf")
                    nc.vector.reciprocal_approx_fast(out=qshf[:], in_=nsh[:])
                    qsh = ang.tile([128, K, S], F16, tag="qsh", name="qsh")
                    nc.vector.tensor_copy(out=qsh[:], in_=qshf[:])
                    nc.vector.tensor_mul(out=a1[:], in0=ta1[:], in1=qreph[:])
                    nc.vector.tensor_mul(out=a2[:], in0=ta2[:], in1=qsh[:])
                    a3q = ang.tile([128, K, S], F16, tag="a3q", name="a3q")
                    for g in range(K):
                        nc.vector.scalar_tensor_tensor(out=a3q[:, g, :], in0=t5[:, g, :],
                                                       scalar=b3[n][:, g:g + 1],
                                                       in1=qreph[:, g, :],
                                                       op0=OP.add, op1=OP.mult)
                    nc.vector.tensor_mul(out=a3[:], in0=a3q[:], in1=qsh[:])
                    a_out[0][n], a_out[1][n], a_out[2][n] = a1, a2, a3
                for a in range(3):
                    dd = angb.tile([128, K, S], F16, tag="dd", name="dd")
                    nc.vector.tensor_sub(out=dd[:], in0=a_out[a][0][:],
                                         in1=a_out[a][1][:])
                    ddump = angb.tile([128, K, S], F16, tag="ddump", name="ddump")
                    nc.scalar.activation(ddump[:], dd[:], AF.Abs,
                                         accum_out=acc[:, 3 * f + a:3 * f + a + 1])
        nc.sync.dma_start(d['acc'][:], acc[:])


# ======================= host side =======================

EXTRA_FRAMES = (1, 3, 5, 7)
SHARED_TEACHER = (2, 4, 6)
SHARED_STUDENT = (1, 2, 3)


def host_prep(teacher_feats, student_feats, ref_perm, shared_perm, n_cores=8):
    t = np.ascontiguousarray(np.asarray(teacher_feats, dtype=np.float32))
    s = np.ascontiguousarray(np.asarray(student_feats, dtype=np.float32))
    rp = np.asarray(ref_perm).astype(np.int64)
    sp = np.asarray(shared_perm).astype(np.int64)
    r4 = (np.arange(RK)[None, :] // K == np.arange(128)[:, None]).astype(np.float32)
    p = np.arange(128)[:, None]
    g = np.arange(K)[None, :]
    rkmap = (g * 128 + p) // K
    in_maps = []
    for c in range(n_cores):
        b, half = c // 2, c % 2
        rows = rp[half * R:(half + 1) * R]
        ref2 = np.stack([t[b, 0][rows], s[b, 0][rows]])           # [2, R, D]
        extra = t[b][list(EXTRA_FRAMES)].reshape(E, D)
        en = extra / np.maximum(np.linalg.norm(extra, axis=1, keepdims=True), 1e-12)
        reftr = np.stack([np.ascontiguousarray(ref2[0].T),
                          np.ascontiguousarray(ref2[1].T)])       # [2, D, R]
        rrv = (ref2.astype(np.float64) ** 2).sum(-1).astype(np.float32)
        sh_bf = np.zeros((NF, D, 2 * S), dtype=ml_dtypes.bfloat16)
        w_rs = np.zeros((NF, 2, 128, S), dtype=np.float32)
        for f in range(NF):
            sh_t = t[b, SHARED_TEACHER[f]][sp]
            sh_s = s[b, SHARED_STUDENT[f]][sp]
            sh_bf[f, :, :S] = sh_t.T
            sh_bf[f, :, S:] = sh_s.T
            ss_t = (sh_t.astype(np.float64) ** 2).sum(-1)
            ss_s = (sh_s.astype(np.float64) ** 2).sum(-1)
            w_rs[f, 0] = rrv[0][:, None] + ss_t[None, :]
            w_rs[f, 1] = rrv[1][:, None] + ss_s[None, :]
        refn = np.repeat(ref2, K, axis=1).astype(ml_dtypes.bfloat16)
        rr_rk = np.stack([rrv[0][rkmap], rrv[1][rkmap]])
        in_maps.append(dict(
            extra_nt=np.ascontiguousarray(en.T),
            reft_t=reftr[0],
            reft_bf=reftr.astype(ml_dtypes.bfloat16),
            sh_bf=sh_bf,
            w_rs=w_rs,
            extra_bf=extra.astype(ml_dtypes.bfloat16),
            refn_bf=refn,
            rr=rr_rk.astype(np.float32),
            r4=r4,
            r4_bf=r4.astype(ml_dtypes.bfloat16),
        ))
    return in_maps


def host_finish(results, B=4):
    total = 0.0
    for r in results:
        total += float(np.asarray(r["acc"], dtype=np.float64).sum())
    denom = NF * B * 256 * S * K
    return np.array(total / denom, dtype=np.float32)


# ======================= self-contained entry =======================

_NC_CACHE = {}


def kernel(teacher_feats, student_feats, ref_perm, shared_perm):
    """Full-input entry: shards across 8 NeuronCores, returns scalar loss (np.float32)."""
    inputs = dict(teacher_feats=np.asarray(teacher_feats),
                  student_feats=np.asarray(student_feats),
                  ref_perm=np.asarray(ref_perm),
                  shared_perm=np.asarray(shared_perm))
    n_cores = 8
    if 'nc' not in _NC_CACHE:
        _NC_CACHE['nc'] = build_program(n_cores=n_cores)
    nc = _NC_CACHE['nc']
    in_maps = host_prep(**inputs, n_cores=n_cores)
    res = run_bass_kernel_spmd(nc, in_maps, core_ids=list(range(n_cores)))
    return host_finish(res.results, B=int(inputs['teacher_feats'].shape[0]))



# revision 12
# speedup vs baseline: 1.2362x; 1.2362x over previous
"""DA3CrossFrameRKDAngleLoss Trainium2 kernel (bass/Tile).  v5

Sharding: 8 cores = (batch b = core//2) x (ref-row half = core%2).
Each core handles R=128 ref rows of one batch; host sums partial sums.

v5 changes vs v4 (247 us):
  - fp16 everywhere on device (full-rate PE, half the HBM traffic of f32r)
  - sim: kc-outer loop, full-PSUM [128,4096] accumulation, 8 big DMAs
  - chunked max8 during last-kc copies; f16 topk
  - the two gpsimd gathers on different DMA queues
  - sr'/reps fill the topk+gather PE-idle window; per-(f,n) P1 psum
    double-buffered so angles (DVE/ACT) overlap next P1 (PE)

Per-core math (R=128, S=256, K=4, D=1024, E=4096, RK=512; rk = g*128+p):
  sim[r,e] = ref_t[r] . extra_unit[e]   fp16 (row scale irrelevant to topk)
  top4 per row -> sim_high[rk] = extra[idx[rk]]  (raw rows, fp16 gather)
  rr[rk] (host)   rh[rk] = ref.sim   hh[rk] = |sim|^2   (device, DVE)
  sr'[r,(f,s)] = ref.shared          (fp16 MMs per net, frames merged)
  T[r,s]   = -2*sr' + (rr+ss)        (DVE stt fold, W=rr+ss uploaded)
  T_rep, q_rep: f16 R4 matmuls -> fp16 SBUF copies
  P1[rk,s] = sim.shared - sr'_rep    (fp16 MMs, per (f,n) psum)
  a1 = (P1 + rr - rh) * q_hr * q_sr
  a2 = (-P1 + hh - rh) * q_hr * q_sh;  n_sh^2 = T_rep - 2*P1 + (hh - rr)
  a3 = (T_rep - P1 + rh - rr) * q_sr * q_sh
  acc[f,a] = sum |a_teacher - a_student|
loss = sum(acc over all cores) / (3*B*256*256*4)
"""
import sys
sys.path.insert(0, '/opt/trn_rl_repo')
import numpy as np
import ml_dtypes

import concourse.bass as bass
import concourse.mybir as mybir
import concourse.tile as tile
from concourse import bacc
from concourse.bass_utils import run_bass_kernel_spmd

AF = mybir.ActivationFunctionType
OP = mybir.AluOpType
F32 = mybir.dt.float32
F16 = mybir.dt.float16

R, S, K, D, E = 128, 256, 4, 1024, 4096
RK = R * K
NF = 3
KC = D // 128          # 8
EC = E // 512          # 8


def build_program(n_cores=8):
    nc = bacc.Bacc("TRN2", target_bir_lowering=False, debug=False,
                   num_devices=n_cores, num_swdge_queues=2)
    d = {}
    d['extra_nt'] = nc.dram_tensor("extra_nt", [D, E], F16, kind="ExternalInput").ap()
    d['reft_t'] = nc.dram_tensor("reft_t", [D, R], F16, kind="ExternalInput").ap()
    d['ref_sr'] = nc.dram_tensor("ref_sr", [2, D, R], F16, kind="ExternalInput").ap()
    d['sh_f'] = nc.dram_tensor("sh_f", [NF, D, 2 * S], F16, kind="ExternalInput").ap()
    d['w_rs'] = nc.dram_tensor("w_rs", [NF, 2, 128, S], F32, kind="ExternalInput").ap()
    d['extra_g'] = nc.dram_tensor("extra_g", [E, D], F16, kind="ExternalInput").ap()
    d['refn'] = nc.dram_tensor("refn", [2, RK, D], F16, kind="ExternalInput").ap()
    d['rr'] = nc.dram_tensor("rr", [2, 128, K], F32, kind="ExternalInput").ap()
    d['r4'] = nc.dram_tensor("r4", [128, RK], F16, kind="ExternalInput").ap()
    d['acc'] = nc.dram_tensor("acc", [128, NF * 3], F32, kind="ExternalOutput").ap()
    d['idx'] = nc.dram_tensor("idx", [128, 8], mybir.dt.uint32, kind="ExternalOutput").ap()

    with tile.TileContext(nc) as tc:
        _body(nc, tc, d)
    nc.compile()
    return nc


def _body(nc, tc, d):
    from contextlib import ExitStack
    with ExitStack() as ctx:
        sb = ctx.enter_context(tc.tile_pool(name="persist", bufs=1))

        # ---- resident tiles ----
        sh = [sb.tile([128, KC, 2 * S], F16, tag=f"sh{f}", name=f"sh{f}") for f in range(NF)]
        w_rs = [[sb.tile([128, S], F32, tag=f"w{f}{n}", name=f"w{f}{n}")
                 for n in range(2)] for f in range(NF)]
        refn = [sb.tile([128, K, D], F16, tag=f"refn{n}", name=f"refn{n}") for n in range(2)]
        rr = [sb.tile([128, K], F32, tag=f"rr{n}", name=f"rr{n}") for n in range(2)]
        r4 = sb.tile([128, RK], F16, tag="r4", name="r4")
        sim_hi = sb.tile([128, K, D], F16, tag="sim_hi", name="sim_hi")
        simT = sb.tile([128, KC, RK], F16, tag="simT", name="simT")
        T_sb = [[sb.tile([128, S], F16, tag=f"T{f}{n}", name=f"T{f}{n}")
                 for n in range(2)] for f in range(NF)]
        qsr_sb = [[sb.tile([128, S], F16, tag=f"qsr{f}{n}", name=f"qsr{f}{n}")
                   for n in range(2)] for f in range(NF)]
        nsrp = [sb.tile([128, 2, S], F16, tag=f"nsrp{f}", name=f"nsrp{f}") for f in range(NF)]
        treph = [[sb.tile([128, K, S], F16, tag=f"treph{f}{n}", name=f"treph{f}{n}")
                  for n in range(2)] for f in range(NF)]
        qreph = [[sb.tile([128, K, S], F16, tag=f"qreph{f}{n}", name=f"qreph{f}{n}")
                  for n in range(2)] for f in range(NF)]
        acc = sb.tile([128, NF * 3], F32, tag="acc", name="acc")

        # bulk loads on scalar/vector queues, ordered by first use
        # bulk loads: scalar queue gets refb+sh (needed ~when sim ends);
        # sync queue gets the rest after the ext chunks (emitted below)
        refb = [sb.tile([128, KC, R], F16, tag=f"refb{n}", name=f"refb{n}") for n in range(2)]
        for n in range(2):
            nc.scalar.dma_start(refb[n][:], d['ref_sr'][n].rearrange("(c p) r -> p c r", p=128))
        for f in range(NF):
            nc.scalar.dma_start(sh[f][:], d['sh_f'][f].rearrange("(c p) s -> p c s", p=128))

        def late_bulk_loads():
            nc.sync.dma_start(r4[:], d['r4'])
            for f in range(NF):
                for n in range(2):
                    nc.sync.dma_start(w_rs[f][n][:], d['w_rs'][f, n])
            for n in range(2):
                nc.sync.dma_start(rr[n][:], d['rr'][n])
                nc.sync.dma_start(refn[n][:], d['refn'][n].rearrange("(g p) x -> p g x", p=128))

        with tc.tile_pool(name="early", bufs=1) as eb:
            reft = eb.tile([128, KC, R], F16, tag="reft", name="reft")
            nc.sync.dma_start(reft[:], d['reft_t'].rearrange("(c p) r -> p c r", p=128))
            sim_sb = eb.tile([128, E], F16, tag="sim_sb", name="sim_sb")
            cmx = eb.tile([128, K, 8], F16, tag="cmx", name="cmx")

            # ---- phase 1: sim, kc-outer, full-PSUM accumulation ----
            with tc.tile_pool(name="ext", bufs=3) as extp, \
                 tc.tile_pool(name="simps", bufs=1, space="PSUM") as simps:
                sim_ps = simps.tile([128, E], F32, tag="sim_ps", name="sim_ps")
                for kc in range(KC):
                    x = extp.tile([128, E], F16, tag="ext", name="ext")
                    nc.sync.dma_start(x[:, 0:E // 2],
                                      d['extra_nt'][kc * 128:(kc + 1) * 128, 0:E // 2])
                    nc.scalar.dma_start(x[:, E // 2:E],
                                        d['extra_nt'][kc * 128:(kc + 1) * 128, E // 2:E])
                    last = kc == KC - 1
                    for e in range(EC):
                        nc.tensor.matmul(sim_ps[:, e * 512:(e + 1) * 512],
                                         reft[:, kc, :], x[:, e * 512:(e + 1) * 512],
                                         start=(kc == 0), stop=last)
                        if last:
                            eng = (nc.scalar, nc.vector)[e % 2]
                            if eng is nc.scalar:
                                nc.scalar.copy(sim_sb[:, e * 512:(e + 1) * 512],
                                               sim_ps[:, e * 512:(e + 1) * 512])
                            else:
                                nc.vector.tensor_copy(out=sim_sb[:, e * 512:(e + 1) * 512],
                                                      in_=sim_ps[:, e * 512:(e + 1) * 512])
                            if e % 2 == 1:
                                nc.vector.max(out=cmx[:, e // 2, :],
                                              in_=sim_sb[:, (e - 1) * 512:(e + 1) * 512])
                late_bulk_loads()

            # ---- phase 1b: sr' per net (PE fills while DVE finishes topk) ----
            sp3 = [None] * 2
            with tc.tile_pool(name="srps", bufs=2, space="PSUM") as srps:
                for n in range(2):
                    sp3[n] = srps.tile([128, NF, S], F32, tag="sp3", name=f"sp3_{n}")
                    for f in range(NF):
                        for kc in range(KC):
                            nc.tensor.matmul(sp3[n][:, f, :], refb[n][:, kc, :],
                                             sh[f][:, kc, n * S:(n + 1) * S],
                                             start=(kc == 0), stop=(kc == KC - 1))

                # ---- phase 2: topk + gathers ----
                mx = eb.tile([128, 8], F16, tag="mx", name="mx")
                mi = eb.tile([128, 8], mybir.dt.uint32, tag="mi", name="mi")
                nc.vector.max(out=mx[:], in_=cmx[:].rearrange("p a b -> p (a b)"))
                nc.vector.max_index(out=mi[:], in_max=mx[:], in_values=sim_sb[:])
                nc.sync.dma_start(d['idx'][:], mi[:])
                idx16 = eb.tile([128, K], mybir.dt.int16, tag="idx16", name="idx16")
                nc.vector.tensor_copy(out=idx16[:], in_=mi[:, 0:K])

                with tc.tile_pool(name="dram", bufs=1, space="DRAM") as drp:
                    idx_dram = drp.tile([RK], mybir.dt.int16, name="idx_dram")
                    nc.sync.dma_start(idx_dram[:].rearrange("(p a) -> p a", p=128),
                                      idx16[:])
                    idxw = eb.tile([128, RK // 16], mybir.dt.int16, tag="idxw", name="idxw")
                    wrapped = idx_dram[:].rearrange("(j q) -> q j", q=16)
                    for sg in range(8):
                        eng = (nc.sync, nc.scalar)[sg % 2]
                        eng.dma_start(idxw[16 * sg:16 * (sg + 1), :], wrapped)
                    nc.gpsimd.dma_gather(simT[:], d['extra_g'], idxw[:], RK, RK, D,
                                         transpose=True, queue_num=0)
                    nc.gpsimd.dma_gather(sim_hi[:], d['extra_g'], idxw[:], RK, RK, D,
                                         queue_num=1)

                # ---- phase 2b: T fold, q_sr, nsrp (from sp3 psum) ----
                for n in range(2):
                    for f in range(NF):
                        nc.vector.scalar_tensor_tensor(out=T_sb[f][n][:],
                                                       in0=sp3[n][:, f, :], scalar=-2.0,
                                                       in1=w_rs[f][n][:],
                                                       op0=OP.mult, op1=OP.add)
                        nsr = eb.tile([128, S], F32, tag="nsr_tmp", name="nsr_tmp")
                        nc.scalar.activation(nsr[:], T_sb[f][n][:], AF.Sqrt, bias=0.0)
                        qtmp = eb.tile([128, S], F32, tag="q_tmp", name="q_tmp")
                        nc.vector.reciprocal_approx_fast(out=qtmp[:], in_=nsr[:])
                        nc.vector.tensor_copy(out=qsr_sb[f][n][:], in_=qtmp[:])
                        nc.scalar.activation(nsrp[f][:, n, :], sp3[n][:, f, :],
                                             AF.Copy, scale=-1.0)

            # ---- phase 3: T_rep / q_rep replication matmuls (fill gather window) ----
            with tc.tile_pool(name="repps", bufs=2, space="PSUM") as repp:
                for f in range(NF):
                    for n in range(2):
                        trep = repp.tile([128, K, S], F32, tag="rep", name="trep")
                        for g in range(K):
                            nc.tensor.matmul(trep[:, g, :], r4[:, g * 128:(g + 1) * 128],
                                             T_sb[f][n][:], start=True, stop=True)
                        nc.scalar.copy(treph[f][n][:], trep[:])
                        qrep = repp.tile([128, K, S], F32, tag="rep", name="qrep")
                        for g in range(K):
                            nc.tensor.matmul(qrep[:, g, :], r4[:, g * 128:(g + 1) * 128],
                                             qsr_sb[f][n][:], start=True, stop=True)
                        nc.scalar.copy(qreph[f][n][:], qrep[:])

        # ---- phase 4: per-(rk) scalars (DVE; needs sim_hi) ----
        hh = sb.tile([128, K], F32, tag="hh", name="hh")
        dump = sb.tile([128, D], F16, tag="dump", name="dump")
        for g in range(K):
            nc.vector.scalar_tensor_tensor(out=dump[:], in0=sim_hi[:, g, :],
                                           scalar=0.0, in1=sim_hi[:, g, :],
                                           op0=OP.bypass, op1=OP.mult,
                                           accum_out=hh[:, g:g + 1])
        rh = [sb.tile([128, K], F32, tag=f"rh{n}", name=f"rh{n}") for n in range(2)]
        for n in range(2):
            for g in range(K):
                nc.vector.scalar_tensor_tensor(out=dump[:], in0=sim_hi[:, g, :],
                                               scalar=0.0, in1=refn[n][:, g, :],
                                               op0=OP.bypass, op1=OP.mult,
                                               accum_out=rh[n][:, g:g + 1])
        b1 = [None] * 2; b2 = [None] * 2; b3 = [None] * 2; bsh = [None] * 2
        qhr = [None] * 2; qhrn = [None] * 2
        for n in range(2):
            u1 = sb.tile([128, K], F32, tag=f"u1{n}", name=f"u1{n}")
            u2 = sb.tile([128, K], F32, tag=f"u2{n}", name=f"u2{n}")
            nc.vector.tensor_sub(out=u1[:], in0=rr[n][:], in1=rh[n][:])
            nc.vector.tensor_sub(out=u2[:], in0=hh[:], in1=rh[n][:])
            nhr = sb.tile([128, K], F32, tag=f"nhr{n}", name=f"nhr{n}")
            nc.vector.tensor_add(out=nhr[:], in0=u1[:], in1=u2[:])
            nc.scalar.activation(nhr[:], nhr[:], AF.Sqrt, bias=0.0)
            qhr[n] = sb.tile([128, K], F32, tag=f"qhr{n}", name=f"qhr{n}")
            nc.vector.reciprocal_approx_fast(out=qhr[n][:], in_=nhr[:])
            qhrn[n] = sb.tile([128, K], F32, tag=f"qhrn{n}", name=f"qhrn{n}")
            nc.vector.tensor_scalar_mul(qhrn[n][:], qhr[n][:], -1.0)
            b1[n] = sb.tile([128, K], F32, tag=f"b1{n}", name=f"b1{n}")
            nc.vector.tensor_mul(out=b1[n][:], in0=u1[:], in1=qhr[n][:])
            b2[n] = sb.tile([128, K], F32, tag=f"b2{n}", name=f"b2{n}")
            nc.vector.tensor_mul(out=b2[n][:], in0=u2[:], in1=qhr[n][:])
            b3[n] = sb.tile([128, K], F32, tag=f"b3{n}", name=f"b3{n}")
            nc.vector.tensor_scalar_mul(b3[n][:], u1[:], -1.0)
            bsh[n] = sb.tile([128, K], F32, tag=f"bsh{n}", name=f"bsh{n}")
            nc.vector.tensor_sub(out=bsh[n][:], in0=u2[:], in1=u1[:])

        # ---- phases 5-6: P1 + angles, pipelined per (f, n) ----
        with tc.tile_pool(name="p1ps", bufs=2, space="PSUM") as p1p, \
             tc.tile_pool(name="angf", bufs=2) as angf, \
             tc.tile_pool(name="ang", bufs=2) as ang, \
             tc.tile_pool(name="aout", bufs=1) as aoutp, \
             tc.tile_pool(name="angb", bufs=2) as angb:
            for f in range(NF):
                a_out = [[None] * 2 for _ in range(3)]
                p1f = p1p.tile([128, K, 2 * S], F32, tag="p1", name="p1")
                for g in range(K):
                    nc.tensor.matmul(p1f[:, g, :], r4[:, g * 128:(g + 1) * 128],
                                     nsrp[f][:].rearrange("p a b -> p (a b)"),
                                     start=True, stop=False)
                for kc in range(KC):
                    for g in range(K):
                        nc.tensor.matmul(p1f[:, g, :],
                                         simT[:, kc, g * 128:(g + 1) * 128],
                                         sh[f][:, kc, :],
                                         start=False, stop=(kc == KC - 1))
                for n in range(2):
                    p1v = p1f[:, :, n * S:(n + 1) * S]        # [128, K, S] psum view
                    # ta1/ta2 per-g (per-partition scale/bias)
                    ta1 = ang.tile([128, K, S], F16, tag="ta1", name="ta1")
                    ta2 = ang.tile([128, K, S], F16, tag="ta2", name="ta2")
                    for g in range(K):
                        nc.scalar.activation(ta1[:, g, :], p1f[:, g, n * S:(n + 1) * S],
                                             AF.Identity, scale=qhr[n][:, g:g + 1],
                                             bias=b1[n][:, g:g + 1])
                        b2_bc = b2[n][:, g:g + 1].broadcast_to([128, S])
                        nc.vector.scalar_tensor_tensor(out=ta2[:, g, :],
                                                       in0=p1f[:, g, n * S:(n + 1) * S],
                                                       scalar=qhrn[n][:, g:g + 1],
                                                       in1=b2_bc, op0=OP.mult, op1=OP.add)
                    t6 = ang.tile([128, K, S], F16, tag="t6", name="t6")
                    nc.vector.scalar_tensor_tensor(out=t6[:], in0=p1v,
                                                   scalar=-2.0, in1=treph[f][n][:],
                                                   op0=OP.mult, op1=OP.add)
                    t5 = ang.tile([128, K, S], F16, tag="t5", name="t5")
                    nc.vector.scalar_tensor_tensor(out=t5[:], in0=p1v,
                                                   scalar=-1.0, in1=treph[f][n][:],
                                                   op0=OP.mult, op1=OP.add)
                    # qsh = 1/sqrt(t6 + bsh)
                    nsh = angf.tile([128, K, S], F32, tag="nsh", name="nsh")
                    for g in range(K):
                        nc.scalar.activation(nsh[:, g, :], t6[:, g, :], AF.Sqrt,
                                             bias=bsh[n][:, g:g + 1])
                    qshf = angf.tile([128, K, S], F32, tag="qshf", name="qshf")
                    nc.vector.reciprocal_approx_fast(out=qshf[:], in_=nsh[:])
                    qsh = ang.tile([128, K, S], F16, tag="qsh", name="qsh")
                    nc.vector.tensor_copy(out=qsh[:], in_=qshf[:])
                    a1 = aoutp.tile([128, K, S], F16, tag=f"a1_{n}", name=f"a1_{n}")
                    a2 = aoutp.tile([128, K, S], F16, tag=f"a2_{n}", name=f"a2_{n}")
                    a3 = aoutp.tile([128, K, S], F16, tag=f"a3_{n}", name=f"a3_{n}")
                    nc.vector.tensor_mul(out=a1[:], in0=ta1[:], in1=qreph[f][n][:])
                    nc.vector.tensor_mul(out=a2[:], in0=ta2[:], in1=qsh[:])
                    a3q = ang.tile([128, K, S], F16, tag="a3q", name="a3q")
                    for g in range(K):
                        nc.vector.scalar_tensor_tensor(out=a3q[:, g, :], in0=t5[:, g, :],
                                                       scalar=b3[n][:, g:g + 1],
                                                       in1=qreph[f][n][:, g, :],
                                                       op0=OP.add, op1=OP.mult)
                    nc.vector.tensor_mul(out=a3[:], in0=a3q[:], in1=qsh[:])
                    a_out[0][n], a_out[1][n], a_out[2][n] = a1, a2, a3
                for a in range(3):
                    dd = angb.tile([128, K, S], F16, tag="dd", name="dd")
                    nc.vector.tensor_sub(out=dd[:], in0=a_out[a][0][:],
                                         in1=a_out[a][1][:])
                    ddump = angb.tile([128, K, S], F16, tag="ddump", name="ddump")
                    nc.scalar.activation(ddump[:], dd[:], AF.Abs,
                                         accum_out=acc[:, 3 * f + a:3 * f + a + 1])
        nc.sync.dma_start(d['acc'][:], acc[:])


# ======================= host side =======================

EXTRA_FRAMES = (1, 3, 5, 7)
SHARED_TEACHER = (2, 4, 6)
SHARED_STUDENT = (1, 2, 3)


def host_prep(teacher_feats, student_feats, ref_perm, shared_perm, n_cores=8):
    t = np.ascontiguousarray(np.asarray(teacher_feats, dtype=np.float32))
    s = np.ascontiguousarray(np.asarray(student_feats, dtype=np.float32))
    rp = np.asarray(ref_perm).astype(np.int64)
    sp = np.asarray(shared_perm).astype(np.int64)
    r4 = (np.arange(RK)[None, :] // K == np.arange(128)[:, None]).astype(np.float16)
    p = np.arange(128)[:, None]
    g = np.arange(K)[None, :]
    rkmap = (g * 128 + p) // K
    in_maps = []
    for c in range(n_cores):
        b, half = c // 2, c % 2
        rows = rp[half * R:(half + 1) * R]
        ref2 = np.stack([t[b, 0][rows], s[b, 0][rows]])           # [2, R, D]
        extra = t[b][list(EXTRA_FRAMES)].reshape(E, D)
        en = extra / np.maximum(np.linalg.norm(extra, axis=1, keepdims=True), 1e-12)
        reftr = np.stack([np.ascontiguousarray(ref2[0].T),
                          np.ascontiguousarray(ref2[1].T)])       # [2, D, R]
        rrv = (ref2.astype(np.float64) ** 2).sum(-1).astype(np.float32)
        sh_f = np.zeros((NF, D, 2 * S), dtype=np.float16)
        w_rs = np.zeros((NF, 2, 128, S), dtype=np.float32)
        for f in range(NF):
            sh_t = t[b, SHARED_TEACHER[f]][sp]
            sh_s = s[b, SHARED_STUDENT[f]][sp]
            sh_f[f, :, :S] = sh_t.T
            sh_f[f, :, S:] = sh_s.T
            ss_t = (sh_t.astype(np.float64) ** 2).sum(-1)
            ss_s = (sh_s.astype(np.float64) ** 2).sum(-1)
            w_rs[f, 0] = rrv[0][:, None] + ss_t[None, :]
            w_rs[f, 1] = rrv[1][:, None] + ss_s[None, :]
        refn = np.repeat(ref2, K, axis=1).astype(np.float16)
        rr_rk = np.stack([rrv[0][rkmap], rrv[1][rkmap]])
        in_maps.append(dict(
            extra_nt=np.ascontiguousarray(en.T).astype(np.float16),
            reft_t=reftr[0].astype(np.float16),
            ref_sr=reftr.astype(np.float16),
            sh_f=sh_f,
            w_rs=w_rs,
            extra_g=extra.astype(np.float16),
            refn=refn,
            rr=rr_rk.astype(np.float32),
            r4=r4,
        ))
    return in_maps


def host_finish(results, B=4):
    total = 0.0
    for r in results:
        total += float(np.asarray(r["acc"], dtype=np.float64).sum())
    denom = NF * B * 256 * S * K
    return np.array(total / denom, dtype=np.float32)


# ======================= self-contained entry =======================

_NC_CACHE = {}


def kernel(teacher_feats, student_feats, ref_perm, shared_perm):
    """Full-input entry: shards across 8 NeuronCores, returns scalar loss (np.float32)."""
    inputs = dict(teacher_feats=np.asarray(teacher_feats),
                  student_feats=np.asarray(student_feats),
                  ref_perm=np.asarray(ref_perm),
                  shared_perm=np.asarray(shared_perm))
    n_cores = 8
    if 'nc' not in _NC_CACHE:
        _NC_CACHE['nc'] = build_program(n_cores=n_cores)
    nc = _NC_CACHE['nc']
    in_maps = host_prep(**inputs, n_cores=n_cores)
    res = run_bass_kernel_spmd(nc, in_maps, core_ids=list(range(n_cores)))
    return host_finish(res.results, B=int(inputs['teacher_feats'].shape[0]))
